# revision 8
# baseline (speedup 1.0000x reference)
"""MHA kernel for trn2: 8 cores = 2 (batch DP) x 4 (head TP, 4 heads/core).

v3: fp16 datapath + ReduceScatter epilogue, chunk-pipelined.
  - All matmul operands fp16 (1 cyc/row on PE vs ~2 for f32r); PSUM fp32.
    Host-emulated numerics: max-rel 6.9e-4 vs fp32 reference (gate 2e-2).
  - x^T [C, T] per batch (host-transposed, partition-relayout for 1-desc
    DMA, first q-chunk split per k-tile so the PE starts ~10us earlier)
  - Q^T/K^T computed as [d, t] via lhsT=W-slice, rhs=x^T; RoPE via PE
    half-swap permutation matmul + signed sin table
  - V computed as [t, d] via lhsT=x^T tile, rhs=Wv (plus ones column for
    softmax denominators)
  - S^T duos [tk=128, 2*512]; 2-head packing on the PE (K=64, base
    partitions 0/64); causal: diagonal blocks get column-restricted
    matmuls + exp + AV (except the accumulation-closing AV); masking via
    one zeros|tri mask multiply per diagonal block (no memsets); AV one
    duo behind S so PE never waits on ACT
  - AV accumulates O_aug^T [65, tq] per head; row 64 = softmax denom
  - denominators scattered to 4 partitions, then 1/d = exp(-ln d) on ACT
    (stays in the natural_log_exp table)
  - epilogue per q-chunk: local partial projection over this core's 256
    head-dims -> full [1024, tq] partial -> fp16 ReduceScatter(add) over
    the 4-core TP group -> DRAM->DRAM copy into the ExternalOutput chunk.
    The PE part (bc broadcast + proj matmuls) is DEFERRED into the next
    chunk's attention stream (after 4 S duos) so the PE never waits on
    the normalize chain, and each RS overlaps the next chunk's compute.
Host reassembles: concat chunk cols, concat group rows, transpose, cast.
"""

import sys

sys.path.insert(0, "/opt/trn_rl_repo")

from contextlib import ExitStack  # noqa: E402

import numpy as np  # noqa: E402

import concourse.bacc as bacc  # noqa: E402
import concourse.bass as bass  # noqa: E402
import concourse.tile as tile  # noqa: E402
from concourse import mybir  # noqa: E402
from concourse.bass_utils import run_bass_kernel_spmd  # noqa: E402

B, T, C, H = 2, 2048, 1024, 16
HD, HD2 = 64, 32
NCORES, GROUPS, HPG, NPAIRS = 8, 4, 4, 2
TK, TQ = 128, 512
NQ = T // TQ  # 4 q-chunks
NKT = T // TK  # 16 tk tiles
KT = C // 128  # 8 contraction tiles
DGRP = 256  # head dims per core (4 heads * 64)
NH = 2 * NPAIRS  # heads per core

F32 = mybir.dt.float32
F16 = mybir.dt.float16
AF = mybir.ActivationFunctionType
ALU = mybir.AluOpType
SCALE = 1.0 / 8.0  # 1/sqrt(HD)


def build_nc():
    nc = bacc.Bacc(target_bir_lowering=False)

    xr = nc.dram_tensor("xr", [128, KT * T], F16, kind="ExternalInput")
    wqr = nc.dram_tensor("wqr", [128, KT * DGRP], F16, kind="ExternalInput")
    wkr = nc.dram_tensor("wkr", [128, KT * DGRP], F16, kind="ExternalInput")
    wvr = nc.dram_tensor("wvr", [128, KT * DGRP], F16, kind="ExternalInput")
    wor = nc.dram_tensor("wor", [128, 2 * C], F16, kind="ExternalInput")
    cos4 = nc.dram_tensor("cos4", [128, T], F16, kind="ExternalInput")
    sin4 = nc.dram_tensor("sin4", [128, T], F16, kind="ExternalInput")
    perm = nc.dram_tensor("perm", [128, 128], F16, kind="ExternalInput")
    # zeros|tri mask: cols [0,384) zero, cols [384,512) upper-tri 128x128
    trim = nc.dram_tensor("trim", [TK, 4 * TK], F16, kind="ExternalInput")
    bq = nc.dram_tensor("bq", [DGRP, 1], F32, kind="ExternalInput")
    bk = nc.dram_tensor("bk", [DGRP, 1], F32, kind="ExternalInput")
    bo = nc.dram_tensor("bo", [C, 1], F32, kind="ExternalInput")
    sel = nc.dram_tensor("sel", [NH, NPAIRS, 128], F16, kind="ExternalInput")
    outs_q = [
        nc.dram_tensor(f"out{q}", [DGRP, TQ], F16, kind="ExternalOutput")
        for q in range(NQ)
    ]

    with tile.TileContext(nc) as tc, ExitStack() as top:
        dram = top.enter_context(tc.tile_pool(name="dram", bufs=1, space="DRAM"))
        y_part_q = [
            dram.tile([KT, 128, TQ], F16, name=f"ypart{q}") for q in range(NQ)
        ]
        # collectives may not write IO tensors; RS lands here, then a small
        # DRAM->DRAM DMA moves each chunk into the ExternalOutput
        y_rs_q = [dram.tile([DGRP, TQ], F16, name=f"yrs{q}") for q in range(NQ)]
        consts = top.enter_context(tc.tile_pool(name="consts", bufs=1))
        cos_sb = consts.tile([128, T], F16)
        sin_sb = consts.tile([128, T], F16)
        perm_sb = consts.tile([128, 128], F16)
        trim_sb = consts.tile([TK, 4 * TK], F16)
        bq_sb = consts.tile([128, NPAIRS], F32)
        bk_sb = consts.tile([128, NPAIRS], F32)
        bo_sb = consts.tile([128, KT], F32)
        sel_sb = consts.tile([NH, NPAIRS, 128], F16)
        bqr = bq.ap().rearrange("(p c) one -> c (p one)", c=128)
        bkr = bk.ap().rearrange("(p c) one -> c (p one)", c=128)
        bor = bo.ap().rearrange("(p c) one -> c (p one)", c=128)

        wo_pool = top.enter_context(tc.tile_pool(name="wo", bufs=1))
        wo_sb = wo_pool.tile([128, 2, C], F16)

        rqk_pool = top.enter_context(tc.tile_pool(name="rqk", bufs=1))
        # RQ/RK per pair: [128, T]; rows = (u1 h0, u2 h0, u1 h1, u2 h1) x 32
        RQ = [rqk_pool.tile([128, T], F16, name=f"RQ{p}") for p in range(NPAIRS)]
        RK = [rqk_pool.tile([128, T], F16, name=f"RK{p}") for p in range(NPAIRS)]
        v_pool = top.enter_context(tc.tile_pool(name="vsb", bufs=1))
        V_sb = v_pool.tile([128, NKT, HPG, HD + 1], F16)

        # ---------------- Phase 1: QKV projection + RoPE ----------------
        with ExitStack() as ph1:
            xt_pool = ph1.enter_context(tc.tile_pool(name="xt", bufs=1))
            xT_sb = xt_pool.tile([128, NQ, KT, TQ], F16)
            wqk_pool = ph1.enter_context(tc.tile_pool(name="wqk", bufs=1))
            wq_sb = wqk_pool.tile([128, KT, DGRP], F16)
            wk_sb = wqk_pool.tile([128, KT, DGRP], F16)
            wv_sb = wqk_pool.tile([128, KT, DGRP], F16)

            # DMA issue order = completion order: feed the PE's phase-1
            # consumption sequence. First q-chunk + wq arrive interleaved
            # per k-tile so the first accumulation chain starts early.
            xrv = xr.ap().rearrange("p (n k t) -> p n k t", n=NQ, k=KT)
            wqv = wqr.ap().rearrange("p (k d) -> p k d", k=KT)
            for k in range(KT):
                nc.gpsimd.dma_start(out=wq_sb[:, k], in_=wqv[:, k])
                nc.gpsimd.dma_start(out=xT_sb[:, 0, k], in_=xrv[:, 0, k])
            nc.gpsimd.dma_start(out=perm_sb, in_=perm.ap())
            nc.gpsimd.dma_start(out=bq_sb, in_=bqr)
            nc.gpsimd.dma_start(out=bk_sb, in_=bkr)
            nc.gpsimd.dma_start(out=xT_sb[:, 1], in_=xrv[:, 1])
            nc.gpsimd.dma_start(out=xT_sb[:, 2], in_=xrv[:, 2])
            nc.gpsimd.dma_start(out=xT_sb[:, 3], in_=xrv[:, 3])
            nc.gpsimd.dma_start(out=wk_sb, in_=wkr.ap())
            nc.gpsimd.dma_start(out=cos_sb, in_=cos4.ap())
            nc.gpsimd.dma_start(out=sin_sb, in_=sin4.ap())
            nc.gpsimd.dma_start(out=wv_sb, in_=wvr.ap())
            nc.gpsimd.dma_start(out=trim_sb, in_=trim.ap())
            nc.gpsimd.dma_start(out=sel_sb, in_=sel.ap())
            nc.gpsimd.dma_start(out=bo_sb, in_=bor)
            nc.gpsimd.dma_start(out=wo_sb, in_=wor.ap())

            ps1 = ph1.enter_context(tc.tile_pool(name="ps1", bufs=2, space="PSUM"))
            tmp_pool = ph1.enter_context(tc.tile_pool(name="tmp", bufs=4))

            def rope_tail(Rc, n):
                # swap 32-row halves via PE perm matmul; sign baked in sin_sb
                sw_ps = ps1.tile([128, TQ], F32, name="sw_ps")
                nc.tensor.matmul(
                    out=sw_ps, lhsT=perm_sb, rhs=Rc, start=True, stop=True
                )
                tmpS = tmp_pool.tile([128, TQ], F16, name="tmpS")
                tmpC = tmp_pool.tile([128, TQ], F16, name="tmpC")
                nc.vector.tensor_mul(tmpS, sw_ps, sin_sb[:, n * TQ : (n + 1) * TQ])
                nc.vector.tensor_mul(tmpC, Rc, cos_sb[:, n * TQ : (n + 1) * TQ])
                nc.vector.tensor_add(Rc, tmpC, tmpS)

            pend = None
            for p in range(NPAIRS):
                for w_sb, b_sb, R in (
                    (wq_sb, bq_sb, RQ[p]),
                    (wk_sb, bk_sb, RK[p]),
                ):
                    for n in range(NQ):
                        u_ps = ps1.tile([128, TQ], F32, name="u_ps")
                        for k in range(KT):
                            nc.tensor.matmul(
                                out=u_ps,
                                lhsT=w_sb[:, k, p * 128 : (p + 1) * 128],
                                rhs=xT_sb[:, n, k, :],
                                start=(k == 0),
                                stop=(k == KT - 1),
                            )
                        Rc = R[:, n * TQ : (n + 1) * TQ]
                        # evict with bias -> R buffer (pre-rotation values)
                        nc.scalar.activation(
                            out=Rc,
                            in_=u_ps,
                            func=AF.Identity,
                            bias=b_sb[:, p : p + 1],
                        )
                        if pend is not None:
                            rope_tail(*pend)
                        pend = (Rc, n)
            rope_tail(*pend)

            # V tiles [t,d] with ones column per head
            nc.vector.memset(V_sb, 1.0)
            for tt in range(NKT):
                v_ps = ps1.tile([128, DGRP], F32, name="v_ps")
                for k in range(KT):
                    nc.tensor.matmul(
                        out=v_ps,
                        lhsT=xT_sb[:, tt // 4, k, (tt % 4) * TK : (tt % 4 + 1) * TK],
                        rhs=wv_sb[:, k, :],
                        start=(k == 0),
                        stop=(k == KT - 1),
                    )
                nc.vector.tensor_copy(
                    out=V_sb[:, tt, :, 0:HD],
                    in_=v_ps.rearrange("p (h d) -> p h d", h=HPG),
                )

        # ---------- Phase 2+3: attention, partial proj + ReduceScatter ----------
        with ExitStack() as ph2:
            sd_pool = ph2.enter_context(tc.tile_pool(name="sduo", bufs=2, space="PSUM"))
            av_pool = ph2.enter_context(tc.tile_pool(name="av", bufs=1, space="PSUM"))
            bc_pool = ph2.enter_context(tc.tile_pool(name="bc", bufs=1, space="PSUM"))
            ps3 = ph2.enter_context(tc.tile_pool(name="ps3", bufs=1, space="PSUM"))
            pt_pool = ph2.enter_context(tc.tile_pool(name="ptile", bufs=3))
            o_pool = ph2.enter_context(tc.tile_pool(name="osb", bufs=2))
            yt_pool = ph2.enter_context(tc.tile_pool(name="yt", bufs=2))
            yp_pool = ph2.enter_context(tc.tile_pool(name="yp", bufs=3))
            dq_pool = ph2.enter_context(tc.tile_pool(name="dq", bufs=2))
            rb_pool = ph2.enter_context(tc.tile_pool(name="rb", bufs=2))
            av = [av_pool.tile([128, TQ], F32, name=f"av{hh}") for hh in range(2)]

            def issue_av(p, qi, ptile, g2, hh):
                last_ti = 4 * qi + 3
                for ji in range(2):
                    ti = 2 * g2 + ji
                    # column-restrict masked diagonal blocks, except the
                    # group-closing matmul (stop flag must cover the tile)
                    off = (
                        TK * (ti - 4 * qi)
                        if (4 * qi < ti < last_ti)
                        else 0
                    )
                    nc.tensor.matmul(
                        out=av[hh][0:65, off:TQ],
                        lhsT=V_sb[:, ti, 2 * p + hh, :],
                        rhs=ptile[:, ji * TQ + off : (ji + 1) * TQ],
                        start=(ti == 0),
                        stop=(ti == last_ti),
                    )

            def emit_epilogue(eqi, o_cur, rbuf):
                # PE part of chunk eqi's epilogue: recip broadcast + partial
                # projection; then evictions, DMA to y_part, ReduceScatter.
                yt2 = yt_pool.tile([128, NPAIRS, TQ], F16, name="yt2")
                for p in range(NPAIRS):
                    bc_ps = bc_pool.tile([128, TQ], F32, name="bc_ps")
                    nc.tensor.matmul(
                        out=bc_ps,
                        lhsT=sel_sb[:, p, :],
                        rhs=rbuf,
                        start=True,
                        stop=True,
                    )
                    nc.vector.tensor_mul(yt2[:, p, :], o_cur[:, p, :], bc_ps)
                y_part = y_part_q[eqi]
                for mb in range(KT):
                    o_ps = ps3.tile([128, TQ], F32, name="o_ps")
                    for p in range(NPAIRS):
                        nc.tensor.matmul(
                            out=o_ps,
                            lhsT=wo_sb[:, p, mb * 128 : (mb + 1) * 128],
                            rhs=yt2[:, p, :],
                            start=(p == 0),
                            stop=(p == NPAIRS - 1),
                        )
                    yp = yp_pool.tile([128, TQ], F16, name="yp")
                    nc.vector.tensor_scalar_add(yp, o_ps, bo_sb[:, mb : mb + 1])
                    nc.sync.dma_start(out=y_part[mb], in_=yp)
                nc.gpsimd.collective_compute(
                    "ReduceScatter",
                    ALU.add,
                    ins=[y_part.opt()],
                    outs=[y_rs_q[eqi].opt()],
                    replica_groups=[[0, 1, 2, 3], [4, 5, 6, 7]],
                )
                nc.sync.dma_start(out=outs_q[eqi].ap(), in_=y_rs_q[eqi].opt())

            pending = None
            for qi in range(NQ):
                o_cur = o_pool.tile([128, NPAIRS, TQ], F32, name="o_cur")
                dq = dq_pool.tile([1, NH, TQ], F16, name="dq")
                for p in range(NPAIRS):
                    avpend = None
                    duos = 0
                    for g2 in range(2 * qi + 2):
                        for hh in range(2):
                            s_ps = sd_pool.tile([128, 2 * TQ], F32, name="s_ps")
                            diag = g2 >= 2 * qi
                            for ji in range(2):
                                ti = 2 * g2 + ji
                                off = TK * (ti - 4 * qi) if diag and ti > 4 * qi else 0
                                col = ji * TQ
                                nc.tensor.matmul(
                                    out=s_ps[:, col + off : col + TQ],
                                    lhsT=RK[p][
                                        64 * hh : 64 * hh + 64,
                                        ti * TK : (ti + 1) * TK,
                                    ],
                                    rhs=RQ[p][
                                        64 * hh : 64 * hh + 64,
                                        qi * TQ + off : (qi + 1) * TQ,
                                    ],
                                    start=True,
                                    stop=True,
                                )
                            ptile = pt_pool.tile([128, 2 * TQ], F16, name="ptile")
                            if diag:
                                for ji in range(2):
                                    ti = 2 * g2 + ji
                                    off = TK * (ti - 4 * qi)
                                    col = ji * TQ
                                    nc.scalar.activation(
                                        out=ptile[:, col + off : col + TQ],
                                        in_=s_ps[:, col + off : col + TQ],
                                        func=AF.Exp,
                                        scale=SCALE,
                                    )
                                    # zero masked cols + upper-tri the
                                    # diagonal 128-block in one mask mul
                                    # (stale cols [col, col+off) x 0 = 0)
                                    nc.vector.tensor_mul(
                                        ptile[:, col : col + off + TK],
                                        ptile[:, col : col + off + TK],
                                        trim_sb[:, 3 * TK - off : 4 * TK],
                                    )
                            else:
                                nc.scalar.activation(
                                    out=ptile, in_=s_ps, func=AF.Exp, scale=SCALE
                                )
                            if avpend is not None:
                                issue_av(p, qi, *avpend)
                            avpend = (ptile, g2, hh)
                            duos += 1
                            if pending is not None and p == 0 and duos == 4:
                                emit_epilogue(*pending)
                                pending = None
                    issue_av(p, qi, *avpend)

                    # evict O_aug + denominators for this (p, qi)
                    nc.vector.tensor_copy(out=o_cur[0:64, p, :], in_=av[0][0:64, :])
                    nc.vector.tensor_copy(
                        out=o_cur[64:128, p, :], in_=av[1][0:64, :]
                    )
                    # engine partition bases must be 32-aligned, so stage
                    # denom rows in free dim of one partition, scatter via DMA
                    nc.vector.tensor_copy(out=dq[0:1, 2 * p, :], in_=av[0][64:65, :])
                    nc.vector.tensor_copy(
                        out=dq[0:1, 2 * p + 1, :], in_=av[1][64:65, :]
                    )

                # normalize prep: scatter denoms to NH partitions, then
                # 1/d = exp(-ln d) on ACT (stays in the ln+exp table)
                rbuf = rb_pool.tile([NH, TQ], F16, name="rbuf")
                nc.gpsimd.dma_start(out=rbuf, in_=dq[0:1, :, :])
                nc.scalar.activation(out=rbuf, in_=rbuf, func=AF.Ln)
                nc.scalar.activation(out=rbuf, in_=rbuf, func=AF.Exp, scale=-1.0)
                pending = (qi, o_cur, rbuf)
            emit_epilogue(*pending)
    nc.finalize()
    return nc


_NC = None


def _get_nc():
    global _NC
    if _NC is None:
        _NC = build_nc()
    return _NC


def _relay(w):
    # [KT*128, M] -> [128, KT*M] so each partition's DMA line is contiguous
    kt, m = w.shape[0] // 128, w.shape[1]
    return np.ascontiguousarray(
        w.reshape(kt, 128, m).transpose(1, 0, 2).reshape(128, kt * m)
    )


def _relay_x(xb):
    # x^T [C, T] -> [128, NQ*KT*TQ] n-major so each 512-col block is one
    # contiguous-per-partition DMA
    xt = xb.T.reshape(KT, 128, NQ, TQ)
    return np.ascontiguousarray(xt.transpose(1, 2, 0, 3).reshape(128, NQ * KT * TQ))


def _in_maps(x, freqs_cos, freqs_sin, Wqkv, bqkv, Wproj, bproj):
    f16 = np.float16
    x = np.asarray(x, f16)
    Wqkv = np.asarray(Wqkv, f16)
    bqkv = np.asarray(bqkv, np.float32)
    Wproj32 = np.asarray(Wproj, np.float32)
    Wproj = Wproj32.astype(f16)
    bproj = np.asarray(bproj, np.float32)
    cos4 = np.ascontiguousarray(np.tile(np.asarray(freqs_cos, f16).T, (4, 1)))
    sinT = np.asarray(freqs_sin, f16).T  # [32, T]
    sin4 = np.ascontiguousarray(np.tile(np.concatenate([-sinT, sinT], axis=0), (2, 1)))
    perm = np.zeros((128, 128), f16)
    for j in range(128):
        i = j + 32 if (j % 64) < 32 else j - 32
        perm[i, j] = 1.0
    trim = np.zeros((TK, 4 * TK), f16)
    trim[:, 3 * TK :] = np.triu(np.ones((TK, TK), f16))
    sel = np.zeros((NH, NPAIRS, 128), f16)
    for p in range(NPAIRS):
        sel[2 * p, p, 0:64] = 1.0
        sel[2 * p + 1, p, 64:128] = 1.0
    bproj_eff = (bproj + bqkv[2 * C : 3 * C] @ Wproj32) / GROUPS
    maps = []
    for r in range(NCORES):
        b, g = r // GROUPS, r % GROUPS
        sl = slice(DGRP * g, DGRP * (g + 1))
        maps.append(
            {
                "xr": _relay_x(x[b]),
                "wqr": _relay(Wqkv[:, 0 * C :][:, sl]),
                "wkr": _relay(Wqkv[:, 1 * C :][:, sl]),
                "wvr": _relay(Wqkv[:, 2 * C :][:, sl]),
                "wor": _relay(Wproj[sl, :]),
                "cos4": cos4,
                "sin4": sin4,
                "perm": perm,
                "trim": trim,
                "sel": sel,
                "bq": np.ascontiguousarray(
                    bqkv[0 * C : 1 * C][sl], np.float32
                ).reshape(DGRP, 1),
                "bk": np.ascontiguousarray(
                    bqkv[1 * C : 2 * C][sl], np.float32
                ).reshape(DGRP, 1),
                "bo": np.ascontiguousarray(bproj_eff, np.float32).reshape(C, 1),
            }
        )
    return maps


def _assemble(results):
    y = np.empty((B, T, C), np.float32)
    for b in range(B):
        cat = np.concatenate(
            [
                np.concatenate(
                    [
                        np.asarray(results[GROUPS * b + g][f"out{q}"])
                        for q in range(NQ)
                    ],
                    axis=1,
                )
                for g in range(GROUPS)
            ],
            axis=0,
        )
        y[b] = cat.T.astype(np.float32)
    return y


def kernel(**inputs):
    nc = _get_nc()
    res = run_bass_kernel_spmd(nc, _in_maps(**inputs), core_ids=list(range(NCORES)))
    return _assemble(res.results)


def kernel_traced(**inputs):
    import tempfile

    nc = _get_nc()
    tmpdir = tempfile.mkdtemp(prefix="mha_trace_")
    res = run_bass_kernel_spmd(
        nc,
        _in_maps(**inputs),
        core_ids=list(range(NCORES)),
        trace=True,
        trace_cores=list(range(NCORES)),
        tmpdir=tmpdir,
    )
    return _assemble(res.results), res.exec_time_ns, tmpdir


# revision 14
# speedup vs baseline: 1.2395x; 1.2395x over previous
"""MHA kernel for trn2: 8 cores = 2 (batch DP) x 4 (head TP, 4 heads/core).

v3: fp16 datapath + ReduceScatter epilogue, chunk-pipelined.
  - All matmul operands fp16 (1 cyc/row on PE vs ~2 for f32r); PSUM fp32.
    Host-emulated numerics: max-rel 6.9e-4 vs fp32 reference (gate 2e-2).
  - x^T [C, T] per batch (host-transposed, partition-relayout for 1-desc
    DMA, first q-chunk split per k-tile so the PE starts ~10us earlier)
  - Q^T/K^T computed as [d, t] via lhsT=W-slice, rhs=x^T; RoPE via PE
    half-swap permutation matmul + signed sin table
  - V computed as [t, d] via lhsT=x^T tile, rhs=Wv (plus ones column for
    softmax denominators)
  - S^T duos [tk=128, 2*512]; 2-head packing on the PE (K=64, base
    partitions 0/64); causal: diagonal blocks get column-restricted
    matmuls + exp + AV (except the accumulation-closing AV); masking via
    one zeros|tri mask multiply per diagonal block (no memsets); AV one
    duo behind S so PE never waits on ACT
  - AV accumulates O_aug^T [65, tq] per head; row 64 = softmax denom
  - denominators scattered to 4 partitions, then 1/d = exp(-ln d) on ACT
    (stays in the natural_log_exp table)
  - epilogue per q-chunk: local partial projection over this core's 256
    head-dims -> full [1024, tq] partial -> fp16 ReduceScatter(add) over
    the 4-core TP group -> DRAM->DRAM copy into the ExternalOutput chunk.
    The PE part (bc broadcast + proj matmuls) is DEFERRED into the next
    chunk's attention stream (after 4 S duos) so the PE never waits on
    the normalize chain, and each RS overlaps the next chunk's compute.
Host reassembles: concat chunk cols, concat group rows, transpose, cast.
"""

import sys

sys.path.insert(0, "/opt/trn_rl_repo")

from contextlib import ExitStack  # noqa: E402

import numpy as np  # noqa: E402

import concourse.bacc as bacc  # noqa: E402
import concourse.bass as bass  # noqa: E402
import concourse.tile as tile  # noqa: E402
from concourse import mybir  # noqa: E402
from concourse.bass_utils import run_bass_kernel_spmd  # noqa: E402

B, T, C, H = 2, 2048, 1024, 16
HD, HD2 = 64, 32
NCORES, GROUPS, HPG, NPAIRS = 8, 4, 4, 2
TK, TQ = 128, 512
NQ = T // TQ  # 4 q-chunks
NKT = T // TK  # 16 tk tiles
KT = C // 128  # 8 contraction tiles
DGRP = 256  # head dims per core (4 heads * 64)
NH = 2 * NPAIRS  # heads per core

F32 = mybir.dt.float32
F16 = mybir.dt.float16
AF = mybir.ActivationFunctionType
ALU = mybir.AluOpType
SCALE = 1.0 / 8.0  # 1/sqrt(HD)


def build_nc():
    nc = bacc.Bacc(target_bir_lowering=False)

    xr = nc.dram_tensor("xr", [128, KT * T], F16, kind="ExternalInput")
    wqr = nc.dram_tensor("wqr", [128, KT * DGRP], F16, kind="ExternalInput")
    wkr = nc.dram_tensor("wkr", [128, KT * DGRP], F16, kind="ExternalInput")
    wvr = nc.dram_tensor("wvr", [128, KT * DGRP], F16, kind="ExternalInput")
    wor = nc.dram_tensor("wor", [128, 2 * C], F16, kind="ExternalInput")
    cos4 = nc.dram_tensor("cos4", [128, T], F16, kind="ExternalInput")
    sin4 = nc.dram_tensor("sin4", [128, T], F16, kind="ExternalInput")
    perm = nc.dram_tensor("perm", [128, 128], F16, kind="ExternalInput")
    # zeros|tri mask: cols [0,384) zero, cols [384,512) upper-tri 128x128
    trim = nc.dram_tensor("trim", [TK, 4 * TK], F16, kind="ExternalInput")
    bq = nc.dram_tensor("bq", [DGRP, 1], F32, kind="ExternalInput")
    bk = nc.dram_tensor("bk", [DGRP, 1], F32, kind="ExternalInput")
    bo = nc.dram_tensor("bo", [C, 1], F32, kind="ExternalInput")
    sel = nc.dram_tensor("sel", [NH, NPAIRS, 128], F16, kind="ExternalInput")
    outs_q = [
        nc.dram_tensor(f"out{q}", [DGRP, TQ], F16, kind="ExternalOutput")
        for q in range(NQ)
    ]

    with tile.TileContext(nc) as tc, ExitStack() as top:
        dram = top.enter_context(tc.tile_pool(name="dram", bufs=1, space="DRAM"))
        y_part_q = [
            dram.tile([KT, 128, TQ], F16, name=f"ypart{q}") for q in range(NQ)
        ]
        # collectives may not write IO tensors; RS lands here, then a small
        # DRAM->DRAM DMA moves each chunk into the ExternalOutput
        y_rs_q = [dram.tile([DGRP, TQ], F16, name=f"yrs{q}") for q in range(NQ)]
        # tiny warmup collective: resolves the runtime's global CC barrier
        # (and cross-core launch skew) during phase 1 instead of stalling
        # the first real ReduceScatter by ~100us
        warm_in = dram.tile([1, 64], F16, name="warm_in")
        warm_out = dram.tile([4, 64], F16, name="warm_out")
        consts = top.enter_context(tc.tile_pool(name="consts", bufs=1))
        cos_sb = consts.tile([128, T], F16)
        sin_sb = consts.tile([128, T], F16)
        perm_sb = consts.tile([128, 128], F16)
        trim_sb = consts.tile([TK, 4 * TK], F16)
        bq_sb = consts.tile([128, NPAIRS], F32)
        bk_sb = consts.tile([128, NPAIRS], F32)
        bo_sb = consts.tile([128, KT], F32)
        sel_sb = consts.tile([NH, NPAIRS, 128], F16)
        bqr = bq.ap().rearrange("(p c) one -> c (p one)", c=128)
        bkr = bk.ap().rearrange("(p c) one -> c (p one)", c=128)
        bor = bo.ap().rearrange("(p c) one -> c (p one)", c=128)

        wo_pool = top.enter_context(tc.tile_pool(name="wo", bufs=1))
        wo_sb = wo_pool.tile([128, 2, C], F16)

        rqk_pool = top.enter_context(tc.tile_pool(name="rqk", bufs=1))
        # RQ/RK per pair: [128, T]; rows = (u1 h0, u2 h0, u1 h1, u2 h1) x 32
        RQ = [rqk_pool.tile([128, T], F16, name=f"RQ{p}") for p in range(NPAIRS)]
        RK = [rqk_pool.tile([128, T], F16, name=f"RK{p}") for p in range(NPAIRS)]
        v_pool = top.enter_context(tc.tile_pool(name="vsb", bufs=1))
        V_sb = v_pool.tile([128, NKT, HPG, HD + 1], F16)

        # ---------------- Phase 1: QKV projection + RoPE ----------------
        with ExitStack() as ph1:
            xt_pool = ph1.enter_context(tc.tile_pool(name="xt", bufs=1))
            xT_sb = xt_pool.tile([128, NQ, KT, TQ], F16)
            wqk_pool = ph1.enter_context(tc.tile_pool(name="wqk", bufs=1))
            wq_sb = wqk_pool.tile([128, KT, DGRP], F16)
            wk_sb = wqk_pool.tile([128, KT, DGRP], F16)
            wv_sb = wqk_pool.tile([128, KT, DGRP], F16)

            # DMA issue order = completion order: feed the PE's phase-1
            # consumption sequence. First q-chunk + wq arrive interleaved
            # per k-tile so the first accumulation chain starts early.
            warm_sb = consts.tile([1, 64], F16, name="warm_sb")
            nc.gpsimd.memset(warm_sb, 0.0)
            nc.gpsimd.dma_start(out=warm_in, in_=warm_sb)
            nc.gpsimd.collective_compute(
                "AllGather",
                ALU.bypass,
                ins=[warm_in.opt()],
                outs=[warm_out.opt()],
                replica_groups=[[0, 1, 2, 3], [4, 5, 6, 7]],
            )
            xrv = xr.ap().rearrange("p (n k t) -> p n k t", n=NQ, k=KT)
            wqv = wqr.ap().rearrange("p (k d) -> p k d", k=KT)
            for k in range(KT):
                nc.gpsimd.dma_start(out=wq_sb[:, k], in_=wqv[:, k])
                nc.gpsimd.dma_start(out=xT_sb[:, 0, k], in_=xrv[:, 0, k])
            nc.gpsimd.dma_start(out=perm_sb, in_=perm.ap())
            nc.gpsimd.dma_start(out=bq_sb, in_=bqr)
            nc.gpsimd.dma_start(out=bk_sb, in_=bkr)
            nc.gpsimd.dma_start(out=xT_sb[:, 1], in_=xrv[:, 1])
            nc.gpsimd.dma_start(out=xT_sb[:, 2], in_=xrv[:, 2])
            nc.gpsimd.dma_start(out=xT_sb[:, 3], in_=xrv[:, 3])
            nc.gpsimd.dma_start(out=wk_sb, in_=wkr.ap())
            nc.gpsimd.dma_start(out=cos_sb, in_=cos4.ap())
            nc.gpsimd.dma_start(out=sin_sb, in_=sin4.ap())
            nc.gpsimd.dma_start(out=wv_sb, in_=wvr.ap())
            nc.gpsimd.dma_start(out=trim_sb, in_=trim.ap())
            nc.gpsimd.dma_start(out=sel_sb, in_=sel.ap())
            nc.gpsimd.dma_start(out=bo_sb, in_=bor)
            nc.gpsimd.dma_start(out=wo_sb, in_=wor.ap())

            ps1 = ph1.enter_context(tc.tile_pool(name="ps1", bufs=2, space="PSUM"))
            tmp_pool = ph1.enter_context(tc.tile_pool(name="tmp", bufs=4))

            def rope_tail(Rc, n):
                # swap 32-row halves via PE perm matmul; sign baked in sin_sb
                sw_ps = ps1.tile([128, TQ], F32, name="sw_ps")
                nc.tensor.matmul(
                    out=sw_ps, lhsT=perm_sb, rhs=Rc, start=True, stop=True
                )
                tmpS = tmp_pool.tile([128, TQ], F16, name="tmpS")
                tmpC = tmp_pool.tile([128, TQ], F16, name="tmpC")
                nc.vector.tensor_mul(tmpS, sw_ps, sin_sb[:, n * TQ : (n + 1) * TQ])
                nc.vector.tensor_mul(tmpC, Rc, cos_sb[:, n * TQ : (n + 1) * TQ])
                nc.vector.tensor_add(Rc, tmpC, tmpS)

            pend = None
            for p in range(NPAIRS):
                for w_sb, b_sb, R in (
                    (wq_sb, bq_sb, RQ[p]),
                    (wk_sb, bk_sb, RK[p]),
                ):
                    for n in range(NQ):
                        u_ps = ps1.tile([128, TQ], F32, name="u_ps")
                        for k in range(KT):
                            nc.tensor.matmul(
                                out=u_ps,
                                lhsT=w_sb[:, k, p * 128 : (p + 1) * 128],
                                rhs=xT_sb[:, n, k, :],
                                start=(k == 0),
                                stop=(k == KT - 1),
                            )
                        Rc = R[:, n * TQ : (n + 1) * TQ]
                        # evict with bias -> R buffer (pre-rotation values)
                        nc.scalar.activation(
                            out=Rc,
                            in_=u_ps,
                            func=AF.Identity,
                            bias=b_sb[:, p : p + 1],
                        )
                        if pend is not None:
                            rope_tail(*pend)
                        pend = (Rc, n)
            rope_tail(*pend)

            # V tiles [t,d] with ones column per head
            nc.vector.memset(V_sb, 1.0)
            for tt in range(NKT):
                v_ps = ps1.tile([128, DGRP], F32, name="v_ps")
                for k in range(KT):
                    nc.tensor.matmul(
                        out=v_ps,
                        lhsT=xT_sb[:, tt // 4, k, (tt % 4) * TK : (tt % 4 + 1) * TK],
                        rhs=wv_sb[:, k, :],
                        start=(k == 0),
                        stop=(k == KT - 1),
                    )
                nc.vector.tensor_copy(
                    out=V_sb[:, tt, :, 0:HD],
                    in_=v_ps.rearrange("p (h d) -> p h d", h=HPG),
                )

        # ---------- Phase 2+3: attention, partial proj + ReduceScatter ----------
        with ExitStack() as ph2:
            sd_pool = ph2.enter_context(tc.tile_pool(name="sduo", bufs=2, space="PSUM"))
            av_pool = ph2.enter_context(tc.tile_pool(name="av", bufs=1, space="PSUM"))
            bc_pool = ph2.enter_context(tc.tile_pool(name="bc", bufs=1, space="PSUM"))
            ps3 = ph2.enter_context(tc.tile_pool(name="ps3", bufs=1, space="PSUM"))
            pt_pool = ph2.enter_context(tc.tile_pool(name="ptile", bufs=3))
            o_pool = ph2.enter_context(tc.tile_pool(name="osb", bufs=2))
            yt_pool = ph2.enter_context(tc.tile_pool(name="yt", bufs=2))
            yp_pool = ph2.enter_context(tc.tile_pool(name="yp", bufs=3))
            dq_pool = ph2.enter_context(tc.tile_pool(name="dq", bufs=2))
            rb_pool = ph2.enter_context(tc.tile_pool(name="rb", bufs=2))
            av = [av_pool.tile([128, TQ], F32, name=f"av{hh}") for hh in range(2)]

            def issue_av(p, qi, ptile, g2, hh):
                last_ti = 4 * qi + 3
                for ji in range(2):
                    ti = 2 * g2 + ji
                    # column-restrict masked diagonal blocks, except the
                    # group-closing matmul (stop flag must cover the tile)
                    off = (
                        TK * (ti - 4 * qi)
                        if (4 * qi < ti < last_ti)
                        else 0
                    )
                    nc.tensor.matmul(
                        out=av[hh][0:65, off:TQ],
                        lhsT=V_sb[:, ti, 2 * p + hh, :],
                        rhs=ptile[:, ji * TQ + off : (ji + 1) * TQ],
                        start=(ti == 0),
                        stop=(ti == last_ti),
                    )

            def emit_epilogue(eqi, o_cur, rbuf):
                # PE part of chunk eqi's epilogue: recip broadcast + partial
                # projection; then evictions, DMA to y_part, ReduceScatter.
                yt2 = yt_pool.tile([128, NPAIRS, TQ], F16, name="yt2")
                for p in range(NPAIRS):
                    bc_ps = bc_pool.tile([128, TQ], F32, name="bc_ps")
                    nc.tensor.matmul(
                        out=bc_ps,
                        lhsT=sel_sb[:, p, :],
                        rhs=rbuf,
                        start=True,
                        stop=True,
                    )
                    nc.vector.tensor_mul(yt2[:, p, :], o_cur[:, p, :], bc_ps)
                y_part = y_part_q[eqi]
                for mb in range(KT):
                    o_ps = ps3.tile([128, TQ], F32, name="o_ps")
                    for p in range(NPAIRS):
                        nc.tensor.matmul(
                            out=o_ps,
                            lhsT=wo_sb[:, p, mb * 128 : (mb + 1) * 128],
                            rhs=yt2[:, p, :],
                            start=(p == 0),
                            stop=(p == NPAIRS - 1),
                        )
                    yp = yp_pool.tile([128, TQ], F16, name="yp")
                    nc.vector.tensor_scalar_add(yp, o_ps, bo_sb[:, mb : mb + 1])
                    nc.sync.dma_start(out=y_part[mb], in_=yp)
                nc.gpsimd.collective_compute(
                    "ReduceScatter",
                    ALU.add,
                    ins=[y_part.opt()],
                    outs=[y_rs_q[eqi].opt()],
                    replica_groups=[[0, 1, 2, 3], [4, 5, 6, 7]],
                )

            pending = None
            for qi in range(NQ):
                o_cur = o_pool.tile([128, NPAIRS, TQ], F32, name="o_cur")
                dq = dq_pool.tile([1, NH, TQ], F16, name="dq")
                for p in range(NPAIRS):
                    avpend = None
                    duos = 0
                    for g2 in range(2 * qi + 2):
                        for hh in range(2):
                            s_ps = sd_pool.tile([128, 2 * TQ], F32, name="s_ps")
                            diag = g2 >= 2 * qi
                            for ji in range(2):
                                ti = 2 * g2 + ji
                                off = TK * (ti - 4 * qi) if diag and ti > 4 * qi else 0
                                col = ji * TQ
                                nc.tensor.matmul(
                                    out=s_ps[:, col + off : col + TQ],
                                    lhsT=RK[p][
                                        64 * hh : 64 * hh + 64,
                                        ti * TK : (ti + 1) * TK,
                                    ],
                                    rhs=RQ[p][
                                        64 * hh : 64 * hh + 64,
                                        qi * TQ + off : (qi + 1) * TQ,
                                    ],
                                    start=True,
                                    stop=True,
                                )
                            ptile = pt_pool.tile([128, 2 * TQ], F16, name="ptile")
                            if diag:
                                for ji in range(2):
                                    ti = 2 * g2 + ji
                                    off = TK * (ti - 4 * qi)
                                    col = ji * TQ
                                    nc.scalar.activation(
                                        out=ptile[:, col + off : col + TQ],
                                        in_=s_ps[:, col + off : col + TQ],
                                        func=AF.Exp,
                                        scale=SCALE,
                                    )
                                    # zero masked cols + upper-tri the
                                    # diagonal 128-block in one mask mul
                                    # (stale cols [col, col+off) x 0 = 0)
                                    nc.vector.tensor_mul(
                                        ptile[:, col : col + off + TK],
                                        ptile[:, col : col + off + TK],
                                        trim_sb[:, 3 * TK - off : 4 * TK],
                                    )
                            else:
                                nc.scalar.activation(
                                    out=ptile, in_=s_ps, func=AF.Exp, scale=SCALE
                                )
                            if avpend is not None:
                                issue_av(p, qi, *avpend)
                            avpend = (ptile, g2, hh)
                            duos += 1
                            if pending is not None and p == 0 and duos == 4:
                                emit_epilogue(*pending)
                                pending = None
                    issue_av(p, qi, *avpend)

                    # evict O_aug + denominators for this (p, qi)
                    nc.vector.tensor_copy(out=o_cur[0:64, p, :], in_=av[0][0:64, :])
                    nc.vector.tensor_copy(
                        out=o_cur[64:128, p, :], in_=av[1][0:64, :]
                    )
                    # engine partition bases must be 32-aligned, so stage
                    # denom rows in free dim of one partition, scatter via DMA
                    nc.vector.tensor_copy(out=dq[0:1, 2 * p, :], in_=av[0][64:65, :])
                    nc.vector.tensor_copy(
                        out=dq[0:1, 2 * p + 1, :], in_=av[1][64:65, :]
                    )

                # normalize prep: scatter denoms to NH partitions, then
                # 1/d = exp(-ln d) on ACT (stays in the ln+exp table)
                rbuf = rb_pool.tile([NH, TQ], F16, name="rbuf")
                nc.gpsimd.dma_start(out=rbuf, in_=dq[0:1, :, :])
                nc.scalar.activation(out=rbuf, in_=rbuf, func=AF.Ln)
                nc.scalar.activation(out=rbuf, in_=rbuf, func=AF.Exp, scale=-1.0)
                pending = (qi, o_cur, rbuf)
            emit_epilogue(*pending)
            # out-copies last: a copy waiting on its RS must never block
            # later eviction DMAs behind it in the queue
            for qi in range(NQ):
                nc.sync.dma_start(out=outs_q[qi].ap(), in_=y_rs_q[qi].opt())
    nc.finalize()
    return nc


_NC = None


def _get_nc():
    global _NC
    if _NC is None:
        _NC = build_nc()
    return _NC


def _relay(w):
    # [KT*128, M] -> [128, KT*M] so each partition's DMA line is contiguous
    kt, m = w.shape[0] // 128, w.shape[1]
    return np.ascontiguousarray(
        w.reshape(kt, 128, m).transpose(1, 0, 2).reshape(128, kt * m)
    )


def _relay_x(xb):
    # x^T [C, T] -> [128, NQ*KT*TQ] n-major so each 512-col block is one
    # contiguous-per-partition DMA
    xt = xb.T.reshape(KT, 128, NQ, TQ)
    return np.ascontiguousarray(xt.transpose(1, 2, 0, 3).reshape(128, NQ * KT * TQ))


def _in_maps(x, freqs_cos, freqs_sin, Wqkv, bqkv, Wproj, bproj):
    f16 = np.float16
    x = np.asarray(x, f16)
    Wqkv = np.asarray(Wqkv, f16)
    bqkv = np.asarray(bqkv, np.float32)
    Wproj32 = np.asarray(Wproj, np.float32)
    Wproj = Wproj32.astype(f16)
    bproj = np.asarray(bproj, np.float32)
    cos4 = np.ascontiguousarray(np.tile(np.asarray(freqs_cos, f16).T, (4, 1)))
    sinT = np.asarray(freqs_sin, f16).T  # [32, T]
    sin4 = np.ascontiguousarray(np.tile(np.concatenate([-sinT, sinT], axis=0), (2, 1)))
    perm = np.zeros((128, 128), f16)
    for j in range(128):
        i = j + 32 if (j % 64) < 32 else j - 32
        perm[i, j] = 1.0
    trim = np.zeros((TK, 4 * TK), f16)
    trim[:, 3 * TK :] = np.triu(np.ones((TK, TK), f16))
    sel = np.zeros((NH, NPAIRS, 128), f16)
    for p in range(NPAIRS):
        sel[2 * p, p, 0:64] = 1.0
        sel[2 * p + 1, p, 64:128] = 1.0
    bproj_eff = (bproj + bqkv[2 * C : 3 * C] @ Wproj32) / GROUPS
    maps = []
    for r in range(NCORES):
        b, g = r // GROUPS, r % GROUPS
        sl = slice(DGRP * g, DGRP * (g + 1))
        maps.append(
            {
                "xr": _relay_x(x[b]),
                "wqr": _relay(Wqkv[:, 0 * C :][:, sl]),
                "wkr": _relay(Wqkv[:, 1 * C :][:, sl]),
                "wvr": _relay(Wqkv[:, 2 * C :][:, sl]),
                "wor": _relay(Wproj[sl, :]),
                "cos4": cos4,
                "sin4": sin4,
                "perm": perm,
                "trim": trim,
                "sel": sel,
                "bq": np.ascontiguousarray(
                    bqkv[0 * C : 1 * C][sl], np.float32
                ).reshape(DGRP, 1),
                "bk": np.ascontiguousarray(
                    bqkv[1 * C : 2 * C][sl], np.float32
                ).reshape(DGRP, 1),
                "bo": np.ascontiguousarray(bproj_eff, np.float32).reshape(C, 1),
            }
        )
    return maps


def _assemble(results):
    y = np.empty((B, T, C), np.float32)
    for b in range(B):
        cat = np.concatenate(
            [
                np.concatenate(
                    [
                        np.asarray(results[GROUPS * b + g][f"out{q}"])
                        for q in range(NQ)
                    ],
                    axis=1,
                )
                for g in range(GROUPS)
            ],
            axis=0,
        )
        y[b] = cat.T.astype(np.float32)
    return y


def kernel(**inputs):
    nc = _get_nc()
    res = run_bass_kernel_spmd(nc, _in_maps(**inputs), core_ids=list(range(NCORES)))
    return _assemble(res.results)


def kernel_traced(**inputs):
    import tempfile

    nc = _get_nc()
    tmpdir = tempfile.mkdtemp(prefix="mha_trace_")
    res = run_bass_kernel_spmd(
        nc,
        _in_maps(**inputs),
        core_ids=list(range(NCORES)),
        trace=True,
        trace_cores=list(range(NCORES)),
        tmpdir=tmpdir,
    )
    return _assemble(res.results), res.exec_time_ns, tmpdir


# revision 26
# speedup vs baseline: 1.2619x; 1.0181x over previous
"""MHA kernel for trn2: 8 cores = 2 (batch DP) x 4 (head TP, 4 heads/core).

v3: fp16 datapath + ReduceScatter epilogue, chunk-pipelined.
  - All matmul operands fp16 (1 cyc/row on PE vs ~2 for f32r); PSUM fp32.
    Host-emulated numerics: max-rel 6.9e-4 vs fp32 reference (gate 2e-2).
  - x^T [C, T] per batch (host-transposed, partition-relayout for 1-desc
    DMA, first q-chunk split per k-tile so the PE starts ~10us earlier)
  - Q^T/K^T computed as [d, t] via lhsT=W-slice, rhs=x^T; RoPE via PE
    half-swap permutation matmul + signed sin table
  - V computed as [t, d] via lhsT=x^T tile, rhs=Wv (plus ones column for
    softmax denominators)
  - S^T duos [tk=128, 2*512]; 2-head packing on the PE (K=64, base
    partitions 0/64); causal: diagonal blocks get column-restricted
    matmuls + exp + AV (except the accumulation-closing AV); masking via
    one zeros|tri mask multiply per diagonal block (no memsets); AV one
    duo behind S so PE never waits on ACT
  - AV accumulates O_aug^T [65, tq] per head; row 64 = softmax denom
  - denominators scattered to 4 partitions, then 1/d = exp(-ln d) on ACT
    (stays in the natural_log_exp table)
  - epilogue per q-chunk: local partial projection over this core's 256
    head-dims -> full [1024, tq] partial -> fp16 ReduceScatter(add) over
    the 4-core TP group -> DRAM->DRAM copy into the ExternalOutput chunk.
    The PE part (bc broadcast + proj matmuls) is DEFERRED into the next
    chunk's attention stream (after 4 S duos) so the PE never waits on
    the normalize chain, and each RS overlaps the next chunk's compute.
Host reassembles: concat chunk cols, concat group rows, transpose, cast.
"""

import sys

sys.path.insert(0, "/opt/trn_rl_repo")

from contextlib import ExitStack  # noqa: E402

import numpy as np  # noqa: E402

import concourse.bacc as bacc  # noqa: E402
import concourse.bass as bass  # noqa: E402
import concourse.tile as tile  # noqa: E402
from concourse import mybir  # noqa: E402
from concourse.bass_utils import run_bass_kernel_spmd  # noqa: E402

B, T, C, H = 2, 2048, 1024, 16
HD, HD2 = 64, 32
NCORES, GROUPS, HPG, NPAIRS = 8, 4, 4, 2
TK, TQ = 128, 512
NQ = T // TQ  # 4 q-chunks
NKT = T // TK  # 16 tk tiles
KT = C // 128  # 8 contraction tiles
DGRP = 256  # head dims per core (4 heads * 64)
NH = 2 * NPAIRS  # heads per core

F32 = mybir.dt.float32
F32R = mybir.dt.float32r
F16 = mybir.dt.float16
AF = mybir.ActivationFunctionType
ALU = mybir.AluOpType
SCALE = 1.0 / 8.0  # 1/sqrt(HD)


def r32(ap):
    return ap.bitcast(F32R)


def build_nc():
    nc = bacc.Bacc(target_bir_lowering=False)

    xr = nc.dram_tensor("xr", [128, KT * T], F16, kind="ExternalInput")
    wqr = nc.dram_tensor("wqr", [128, KT * DGRP], F16, kind="ExternalInput")
    wkr = nc.dram_tensor("wkr", [128, KT * DGRP], F16, kind="ExternalInput")
    wvr = nc.dram_tensor("wvr", [128, KT * DGRP], F16, kind="ExternalInput")
    wor = nc.dram_tensor("wor", [128, 2 * C], F16, kind="ExternalInput")
    cos4 = nc.dram_tensor("cos4", [128, T], F16, kind="ExternalInput")
    sin4 = nc.dram_tensor("sin4", [128, T], F16, kind="ExternalInput")
    perm = nc.dram_tensor("perm", [128, 128], F16, kind="ExternalInput")
    # zeros|tri mask: cols [0,384) zero, cols [384,512) upper-tri 128x128
    trim = nc.dram_tensor("trim", [TK, 4 * TK], F16, kind="ExternalInput")
    bq = nc.dram_tensor("bq", [DGRP, 1], F32, kind="ExternalInput")
    bk = nc.dram_tensor("bk", [DGRP, 1], F32, kind="ExternalInput")
    bo = nc.dram_tensor("bo", [C, 1], F32, kind="ExternalInput")
    sel = nc.dram_tensor("sel", [NH, NPAIRS, 128], F16, kind="ExternalInput")
    outs_q = [
        nc.dram_tensor(f"out{q}", [DGRP, TQ], F16, kind="ExternalOutput")
        for q in range(NQ)
    ]

    with tile.TileContext(nc) as tc, ExitStack() as top:
        dram = top.enter_context(tc.tile_pool(name="dram", bufs=1, space="DRAM"))
        y_part_q = [
            dram.tile([KT, 128, TQ], F16, name=f"ypart{q}") for q in range(NQ)
        ]
        # collectives may not write IO tensors; RS lands here, then a small
        # DRAM->DRAM DMA moves each chunk into the ExternalOutput
        y_rs_q = [dram.tile([DGRP, TQ], F16, name=f"yrs{q}") for q in range(NQ)]
        # tiny warmup collective: resolves the runtime's global CC barrier
        # (and cross-core launch skew) during phase 1 instead of stalling
        # the first real ReduceScatter by ~100us
        warm_in = dram.tile([1, 64], F16, name="warm_in")
        warm_out = dram.tile([4, 64], F16, name="warm_out")
        consts = top.enter_context(tc.tile_pool(name="consts", bufs=1))
        cos_sb = consts.tile([128, T], F16)
        sin_sb = consts.tile([128, T], F16)
        perm_sb = consts.tile([128, 128], F16)
        trim_sb = consts.tile([TK, 4 * TK], F16)
        bq_sb = consts.tile([128, NPAIRS], F32)
        bk_sb = consts.tile([128, NPAIRS], F32)
        bo_sb = consts.tile([128, KT], F32)
        sel_sb = consts.tile([NH, NPAIRS, 128], F16)
        bqr = bq.ap().rearrange("(p c) one -> c (p one)", c=128)
        bkr = bk.ap().rearrange("(p c) one -> c (p one)", c=128)
        bor = bo.ap().rearrange("(p c) one -> c (p one)", c=128)

        wo_pool = top.enter_context(tc.tile_pool(name="wo", bufs=1))
        wo_sb = wo_pool.tile([128, 2, C], F16)

        rqk_pool = top.enter_context(tc.tile_pool(name="rqk", bufs=1))
        # RQ/RK per pair: [128, T]; rows = (u1 h0, u2 h0, u1 h1, u2 h1) x 32
        RQ = [rqk_pool.tile([128, T], F16, name=f"RQ{p}") for p in range(NPAIRS)]
        RK = [rqk_pool.tile([128, T], F16, name=f"RK{p}") for p in range(NPAIRS)]
        v_pool = top.enter_context(tc.tile_pool(name="vsb", bufs=1))
        V_sb = v_pool.tile([128, NKT, HPG, HD + 1], F16)

        # ---------------- Phase 1: QKV projection + RoPE ----------------
        with ExitStack() as ph1:
            xt_pool = ph1.enter_context(tc.tile_pool(name="xt", bufs=1))
            xT_sb = xt_pool.tile([128, NQ, KT, TQ], F16)
            wqk_pool = ph1.enter_context(tc.tile_pool(name="wqk", bufs=1))
            wq_sb = wqk_pool.tile([128, KT, DGRP], F16)
            wk_sb = wqk_pool.tile([128, KT, DGRP], F16)
            wv_sb = wqk_pool.tile([128, KT, DGRP], F16)

            # DMA issue order = completion order: feed the PE's phase-1
            # consumption sequence. First q-chunk + wq arrive interleaved
            # per k-tile so the first accumulation chain starts early.
            warm_sb = consts.tile([1, 64], F16, name="warm_sb")
            nc.gpsimd.memset(warm_sb, 0.0)
            nc.gpsimd.dma_start(out=warm_in, in_=warm_sb)
            nc.gpsimd.collective_compute(
                "AllGather",
                ALU.bypass,
                ins=[warm_in.opt()],
                outs=[warm_out.opt()],
                replica_groups=[[0, 1, 2, 3], [4, 5, 6, 7]],
            )
            # inputs split across two queues: weights on gpsimd, x on sync,
            # interleaved per k-tile so the first accumulation starts early
            xrv = xr.ap().rearrange("p (n k t) -> p n k t", n=NQ, k=KT)
            wqv = wqr.ap().rearrange("p (k d) -> p k d", k=KT)
            for k in range(KT):
                nc.gpsimd.dma_start(out=wq_sb[:, k], in_=wqv[:, k])
                nc.sync.dma_start(out=xT_sb[:, 0, k], in_=xrv[:, 0, k])
            nc.gpsimd.dma_start(out=perm_sb, in_=perm.ap())
            nc.gpsimd.dma_start(out=bq_sb, in_=bqr)
            nc.gpsimd.dma_start(out=bk_sb, in_=bkr)
            nc.sync.dma_start(out=xT_sb[:, 1], in_=xrv[:, 1])
            nc.gpsimd.dma_start(out=wk_sb, in_=wkr.ap())
            nc.sync.dma_start(out=xT_sb[:, 2], in_=xrv[:, 2])
            nc.sync.dma_start(out=xT_sb[:, 3], in_=xrv[:, 3])
            nc.gpsimd.dma_start(out=cos_sb, in_=cos4.ap())
            nc.gpsimd.dma_start(out=sin_sb, in_=sin4.ap())
            nc.gpsimd.dma_start(out=wv_sb, in_=wvr.ap())
            nc.gpsimd.dma_start(out=trim_sb, in_=trim.ap())
            nc.gpsimd.dma_start(out=sel_sb, in_=sel.ap())
            nc.gpsimd.dma_start(out=bo_sb, in_=bor)
            nc.gpsimd.dma_start(out=wo_sb, in_=wor.ap())

            ps1 = ph1.enter_context(tc.tile_pool(name="ps1", bufs=2, space="PSUM"))
            tmp_pool = ph1.enter_context(tc.tile_pool(name="tmp", bufs=4))

            def rope_tail(Rc, n):
                # swap 32-row halves via PE perm matmul; sign baked in sin_sb
                sw_ps = ps1.tile([128, TQ], F32, name="sw_ps")
                nc.tensor.matmul(
                    out=sw_ps, lhsT=perm_sb, rhs=Rc, start=True, stop=True
                )
                tmpS = tmp_pool.tile([128, TQ], F16, name="tmpS")
                tmpC = tmp_pool.tile([128, TQ], F16, name="tmpC")
                nc.vector.tensor_mul(tmpS, sw_ps, sin_sb[:, n * TQ : (n + 1) * TQ])
                nc.vector.tensor_mul(tmpC, Rc, cos_sb[:, n * TQ : (n + 1) * TQ])
                nc.vector.tensor_add(Rc, tmpC, tmpS)

            pend = None
            for p in range(NPAIRS):
                for w_sb, b_sb, R in (
                    (wq_sb, bq_sb, RQ[p]),
                    (wk_sb, bk_sb, RK[p]),
                ):
                    for n in range(NQ):
                        u_ps = ps1.tile([128, TQ], F32, name="u_ps")
                        for k in range(KT):
                            nc.tensor.matmul(
                                out=u_ps,
                                lhsT=w_sb[:, k, p * 128 : (p + 1) * 128],
                                rhs=xT_sb[:, n, k, :],
                                start=(k == 0),
                                stop=(k == KT - 1),
                            )
                        Rc = R[:, n * TQ : (n + 1) * TQ]
                        # evict with bias -> R buffer (pre-rotation values)
                        nc.scalar.activation(
                            out=Rc,
                            in_=u_ps,
                            func=AF.Identity,
                            bias=b_sb[:, p : p + 1],
                        )
                        if pend is not None:
                            rope_tail(*pend)
                        pend = (Rc, n)
            rope_tail(*pend)

            # V tiles [t,d] with ones column per head
            nc.vector.memset(V_sb, 1.0)
            for tt in range(NKT):
                v_ps = ps1.tile([128, DGRP], F32, name="v_ps")
                for k in range(KT):
                    nc.tensor.matmul(
                        out=v_ps,
                        lhsT=xT_sb[:, tt // 4, k, (tt % 4) * TK : (tt % 4 + 1) * TK],
                        rhs=wv_sb[:, k, :],
                        start=(k == 0),
                        stop=(k == KT - 1),
                    )
                nc.vector.tensor_copy(
                    out=V_sb[:, tt, :, 0:HD],
                    in_=v_ps.rearrange("p (h d) -> p h d", h=HPG),
                )

        # ---------- Phase 2+3: attention, partial proj + ReduceScatter ----------
        with ExitStack() as ph2:
            sd_pool = ph2.enter_context(tc.tile_pool(name="sduo", bufs=4, space="PSUM"))
            av_pool = ph2.enter_context(tc.tile_pool(name="av", bufs=1, space="PSUM"))
            bc_pool = ph2.enter_context(tc.tile_pool(name="bc", bufs=1, space="PSUM"))
            ps3 = ph2.enter_context(tc.tile_pool(name="ps3", bufs=1, space="PSUM"))
            pt_pool = ph2.enter_context(tc.tile_pool(name="ptile", bufs=4))
            o_pool = ph2.enter_context(tc.tile_pool(name="osb", bufs=2))
            yt_pool = ph2.enter_context(tc.tile_pool(name="yt", bufs=2))
            yp_pool = ph2.enter_context(tc.tile_pool(name="yp", bufs=3))
            dq_pool = ph2.enter_context(tc.tile_pool(name="dq", bufs=2))
            rb_pool = ph2.enter_context(tc.tile_pool(name="rb", bufs=2))
            av = [av_pool.tile([128, TQ], F32, name=f"av{hh}") for hh in range(2)]

            def issue_av(p, qi, ptile, ti, hh):
                last_ti = 4 * qi + 3
                # column-restrict masked diagonal blocks, except the
                # group-closing matmul (stop flag must cover the tile)
                off = TK * (ti - 4 * qi) if (4 * qi < ti < last_ti) else 0
                nc.tensor.matmul(
                    out=av[hh][0:65, off:TQ],
                    lhsT=V_sb[:, ti, 2 * p + hh, :],
                    rhs=ptile[:, (ti % 2) * TQ + off : (ti % 2 + 1) * TQ],
                    start=(ti == 0),
                    stop=(ti == last_ti),
                )

            def emit_epilogue(eqi, o_cur, rbuf):
                # PE part of chunk eqi's epilogue: recip broadcast + partial
                # projection; then evictions, DMA to y_part, ReduceScatter.
                yt2 = yt_pool.tile([128, NPAIRS, TQ], F16, name="yt2")
                for p in range(NPAIRS):
                    bc_ps = bc_pool.tile([128, TQ], F32, name="bc_ps")
                    nc.tensor.matmul(
                        out=bc_ps,
                        lhsT=sel_sb[:, p, :],
                        rhs=rbuf,
                        start=True,
                        stop=True,
                    )
                    nc.vector.tensor_mul(yt2[:, p, :], o_cur[:, p, :], bc_ps)
                y_part = y_part_q[eqi]
                for mb in range(KT):
                    o_ps = ps3.tile([128, TQ], F32, name="o_ps")
                    for p in range(NPAIRS):
                        nc.tensor.matmul(
                            out=o_ps,
                            lhsT=wo_sb[:, p, mb * 128 : (mb + 1) * 128],
                            rhs=yt2[:, p, :],
                            start=(p == 0),
                            stop=(p == NPAIRS - 1),
                        )
                    yp = yp_pool.tile([128, TQ], F16, name="yp")
                    nc.vector.tensor_scalar_add(yp, o_ps, bo_sb[:, mb : mb + 1])
                    nc.sync.dma_start(out=y_part[mb], in_=yp)
                nc.gpsimd.collective_compute(
                    "ReduceScatter",
                    ALU.add,
                    ins=[y_part.opt()],
                    outs=[y_rs_q[eqi].opt()],
                    replica_groups=[[0, 1, 2, 3], [4, 5, 6, 7]],
                )

            pending = None
            for qi in range(NQ):
                o_cur = o_pool.tile([128, NPAIRS, TQ], F32, name="o_cur")
                dq = dq_pool.tile([1, NH, TQ], F32, name="dq")
                for p in range(NPAIRS):
                    avq = []
                    units = 0
                    for g2 in range(2 * qi + 2):
                        for hh in range(2):
                            diag = g2 >= 2 * qi
                            ptile = pt_pool.tile([128, 2 * TQ], F16, name="ptile")
                            for ji in range(2):
                                ti = 2 * g2 + ji
                                off = TK * (ti - 4 * qi) if diag and ti > 4 * qi else 0
                                col = ji * TQ
                                s_u = sd_pool.tile([128, TQ], F32, name="s_u")
                                nc.tensor.matmul(
                                    out=s_u[:, off:TQ],
                                    lhsT=RK[p][
                                        64 * hh : 64 * hh + 64,
                                        ti * TK : (ti + 1) * TK,
                                    ],
                                    rhs=RQ[p][
                                        64 * hh : 64 * hh + 64,
                                        qi * TQ + off : (qi + 1) * TQ,
                                    ],
                                    start=True,
                                    stop=True,
                                )
                                nc.scalar.activation(
                                    out=ptile[:, col + off : col + TQ],
                                    in_=s_u[:, off:TQ],
                                    func=AF.Exp,
                                    scale=SCALE,
                                )
                                if diag:
                                    # zero masked cols + upper-tri the
                                    # diagonal 128-block in one mask mul
                                    # (stale cols [col, col+off) x 0 = 0)
                                    nc.vector.tensor_mul(
                                        ptile[:, col : col + off + TK],
                                        ptile[:, col : col + off + TK],
                                        trim_sb[:, 3 * TK - off : 4 * TK],
                                    )
                                avq.append((ptile, ti, hh))
                                # AV lags S by 2 units so the PE never
                                # waits on the exp of its own unit
                                if len(avq) > 2:
                                    issue_av(p, qi, *avq.pop(0))
                                units += 1
                                if pending is not None and p == 0 and units == 8:
                                    emit_epilogue(*pending)
                                    pending = None
                    for u in avq:
                        issue_av(p, qi, *u)

                    # evict O_aug + denominators for this (p, qi)
                    nc.vector.tensor_copy(out=o_cur[0:64, p, :], in_=av[0][0:64, :])
                    nc.vector.tensor_copy(
                        out=o_cur[64:128, p, :], in_=av[1][0:64, :]
                    )
                    # engine partition bases must be 32-aligned, so stage
                    # denom rows in free dim of one partition, scatter via DMA
                    nc.vector.tensor_copy(out=dq[0:1, 2 * p, :], in_=av[0][64:65, :])
                    nc.vector.tensor_copy(
                        out=dq[0:1, 2 * p + 1, :], in_=av[1][64:65, :]
                    )

                # normalize prep: scatter denoms to NH partitions, then
                # 1/d on the DVE (ACT Ln/Exp would thrash the act table:
                # 2x 1.28us reloads per chunk, right in the critical chain)
                rbuf = rb_pool.tile([NH, TQ], F32, name="rbuf")
                rb16 = rb_pool.tile([NH, TQ], F16, name="rb16")
                nc.gpsimd.dma_start(out=rbuf, in_=dq[0:1, :, :])
                nc.vector.reciprocal(out=rbuf, in_=rbuf)
                nc.vector.tensor_copy(out=rb16, in_=rbuf)
                pending = (qi, o_cur, rb16)
            emit_epilogue(*pending)
            # out-copies last: a copy waiting on its RS must never block
            # later eviction DMAs behind it in the queue
            for qi in range(NQ):
                nc.sync.dma_start(out=outs_q[qi].ap(), in_=y_rs_q[qi].opt())
    nc.finalize()
    return nc


_NC = None


def _get_nc():
    global _NC
    if _NC is None:
        _NC = build_nc()
    return _NC


def _relay(w):
    # [KT*128, M] -> [128, KT*M] so each partition's DMA line is contiguous
    kt, m = w.shape[0] // 128, w.shape[1]
    return np.ascontiguousarray(
        w.reshape(kt, 128, m).transpose(1, 0, 2).reshape(128, kt * m)
    )


def _relay_x(xb):
    # x^T [C, T] -> [128, NQ*KT*TQ] n-major so each 512-col block is one
    # contiguous-per-partition DMA
    xt = xb.T.reshape(KT, 128, NQ, TQ)
    return np.ascontiguousarray(xt.transpose(1, 2, 0, 3).reshape(128, NQ * KT * TQ))


def _in_maps(x, freqs_cos, freqs_sin, Wqkv, bqkv, Wproj, bproj):
    f16 = np.float16
    x = np.asarray(x, f16)
    Wqkv = np.asarray(Wqkv, f16)
    bqkv = np.asarray(bqkv, np.float32)
    Wproj32 = np.asarray(Wproj, np.float32)
    Wproj = Wproj32.astype(f16)
    bproj = np.asarray(bproj, np.float32)
    cos4 = np.ascontiguousarray(np.tile(np.asarray(freqs_cos, f16).T, (4, 1)))
    sinT = np.asarray(freqs_sin, f16).T  # [32, T]
    sin4 = np.ascontiguousarray(np.tile(np.concatenate([-sinT, sinT], axis=0), (2, 1)))
    perm = np.zeros((128, 128), f16)
    for j in range(128):
        i = j + 32 if (j % 64) < 32 else j - 32
        perm[i, j] = 1.0
    trim = np.zeros((TK, 4 * TK), f16)
    trim[:, 3 * TK :] = np.triu(np.ones((TK, TK), f16))
    sel = np.zeros((NH, NPAIRS, 128), f16)
    for p in range(NPAIRS):
        sel[2 * p, p, 0:64] = 1.0
        sel[2 * p + 1, p, 64:128] = 1.0
    bproj_eff = (bproj + bqkv[2 * C : 3 * C] @ Wproj32) / GROUPS
    maps = []
    for r in range(NCORES):
        b, g = r // GROUPS, r % GROUPS
        sl = slice(DGRP * g, DGRP * (g + 1))
        maps.append(
            {
                "xr": _relay_x(x[b]),
                "wqr": _relay(Wqkv[:, 0 * C :][:, sl]),
                "wkr": _relay(Wqkv[:, 1 * C :][:, sl]),
                "wvr": _relay(Wqkv[:, 2 * C :][:, sl]),
                "wor": _relay(Wproj[sl, :]),
                "cos4": cos4,
                "sin4": sin4,
                "perm": perm,
                "trim": trim,
                "sel": sel,
                "bq": np.ascontiguousarray(
                    bqkv[0 * C : 1 * C][sl], np.float32
                ).reshape(DGRP, 1),
                "bk": np.ascontiguousarray(
                    bqkv[1 * C : 2 * C][sl], np.float32
                ).reshape(DGRP, 1),
                "bo": np.ascontiguousarray(bproj_eff, np.float32).reshape(C, 1),
            }
        )
    return maps


def _assemble(results):
    y = np.empty((B, T, C), np.float32)
    for b in range(B):
        cat = np.concatenate(
            [
                np.concatenate(
                    [
                        np.asarray(results[GROUPS * b + g][f"out{q}"])
                        for q in range(NQ)
                    ],
                    axis=1,
                )
                for g in range(GROUPS)
            ],
            axis=0,
        )
        y[b] = cat.T.astype(np.float32)
    return y


def kernel(**inputs):
    nc = _get_nc()
    res = run_bass_kernel_spmd(nc, _in_maps(**inputs), core_ids=list(range(NCORES)))
    return _assemble(res.results)


def kernel_traced(**inputs):
    import tempfile

    nc = _get_nc()
    tmpdir = tempfile.mkdtemp(prefix="mha_trace_")
    res = run_bass_kernel_spmd(
        nc,
        _in_maps(**inputs),
        core_ids=list(range(NCORES)),
        trace=True,
        trace_cores=list(range(NCORES)),
        tmpdir=tmpdir,
    )
    return _assemble(res.results), res.exec_time_ns, tmpdir


# revision 35
# speedup vs baseline: 1.2648x; 1.0023x over previous
"""MHA kernel for trn2: 8 cores = 2 (batch DP) x 4 (head TP, 4 heads/core).

v3: fp16 datapath + ReduceScatter epilogue, chunk-pipelined.
  - All matmul operands fp16 (1 cyc/row on PE vs ~2 for f32r); PSUM fp32.
    Host-emulated numerics: max-rel 6.9e-4 vs fp32 reference (gate 2e-2).
  - x^T [C, T] per batch (host-transposed, partition-relayout for 1-desc
    DMA, first q-chunk split per k-tile so the PE starts ~10us earlier)
  - Q^T/K^T computed as [d, t] via lhsT=W-slice, rhs=x^T; RoPE via PE
    half-swap permutation matmul + signed sin table
  - V computed as [t, d] via lhsT=x^T tile, rhs=Wv (plus ones column for
    softmax denominators)
  - S^T duos [tk=128, 2*512]; 2-head packing on the PE (K=64, base
    partitions 0/64); causal: diagonal blocks get column-restricted
    matmuls + exp + AV (except the accumulation-closing AV); masking via
    one zeros|tri mask multiply per diagonal block (no memsets); AV one
    duo behind S so PE never waits on ACT
  - AV accumulates O_aug^T [65, tq] per head; row 64 = softmax denom
  - denominators scattered to 4 partitions, then 1/d = exp(-ln d) on ACT
    (stays in the natural_log_exp table)
  - epilogue per q-chunk: local partial projection over this core's 256
    head-dims -> full [1024, tq] partial -> fp16 ReduceScatter(add) over
    the 4-core TP group -> DRAM->DRAM copy into the ExternalOutput chunk.
    The PE part (bc broadcast + proj matmuls) is DEFERRED into the next
    chunk's attention stream (after 4 S duos) so the PE never waits on
    the normalize chain, and each RS overlaps the next chunk's compute.
Host reassembles: concat chunk cols, concat group rows, transpose, cast.
"""

import sys

sys.path.insert(0, "/opt/trn_rl_repo")

from contextlib import ExitStack  # noqa: E402

import numpy as np  # noqa: E402

import concourse.bacc as bacc  # noqa: E402
import concourse.bass as bass  # noqa: E402
import concourse.tile as tile  # noqa: E402
from concourse import mybir  # noqa: E402
from concourse.bass_utils import run_bass_kernel_spmd  # noqa: E402

B, T, C, H = 2, 2048, 1024, 16
HD, HD2 = 64, 32
NCORES, GROUPS, HPG, NPAIRS = 8, 4, 4, 2
TK, TQ = 128, 512
NQ = T // TQ  # 4 q-chunks
NKT = T // TK  # 16 tk tiles
KT = C // 128  # 8 contraction tiles
DGRP = 256  # head dims per core (4 heads * 64)
NH = 2 * NPAIRS  # heads per core

F32 = mybir.dt.float32
F32R = mybir.dt.float32r
F16 = mybir.dt.float16
AF = mybir.ActivationFunctionType
ALU = mybir.AluOpType
SCALE = 1.0 / 8.0  # 1/sqrt(HD)


def r32(ap):
    return ap.bitcast(F32R)


def build_nc():
    nc = bacc.Bacc(target_bir_lowering=False)

    xr = nc.dram_tensor("xr", [128, KT * T], F16, kind="ExternalInput")
    wqr = nc.dram_tensor("wqr", [128, KT * DGRP], F16, kind="ExternalInput")
    wkr = nc.dram_tensor("wkr", [128, KT * DGRP], F16, kind="ExternalInput")
    wvr = nc.dram_tensor("wvr", [128, KT * DGRP], F16, kind="ExternalInput")
    wor = nc.dram_tensor("wor", [128, 2 * C], F16, kind="ExternalInput")
    cos4 = nc.dram_tensor("cos4", [128, T], F16, kind="ExternalInput")
    sin4 = nc.dram_tensor("sin4", [128, T], F16, kind="ExternalInput")
    perm = nc.dram_tensor("perm", [128, 128], F16, kind="ExternalInput")
    # zeros|tri mask: cols [0,384) zero, cols [384,512) upper-tri 128x128
    trim = nc.dram_tensor("trim", [TK, 4 * TK], F16, kind="ExternalInput")
    bq = nc.dram_tensor("bq", [DGRP, 1], F32, kind="ExternalInput")
    bk = nc.dram_tensor("bk", [DGRP, 1], F32, kind="ExternalInput")
    bo = nc.dram_tensor("bo", [C, 1], F32, kind="ExternalInput")
    sel = nc.dram_tensor("sel", [NH, NPAIRS, 128], F16, kind="ExternalInput")
    outs_q = [
        nc.dram_tensor(f"out{q}", [DGRP, TQ], F16, kind="ExternalOutput")
        for q in range(NQ)
    ]

    with tile.TileContext(nc) as tc, ExitStack() as top:
        dram = top.enter_context(tc.tile_pool(name="dram", bufs=1, space="DRAM"))
        y_part_q = [
            dram.tile([KT, 128, TQ], F16, name=f"ypart{q}") for q in range(NQ)
        ]
        # collectives may not write IO tensors; RS lands here, then a small
        # DRAM->DRAM DMA moves each chunk into the ExternalOutput
        y_rs_q = [dram.tile([DGRP, TQ], F16, name=f"yrs{q}") for q in range(NQ)]
        # tiny warmup collective: resolves the runtime's global CC barrier
        # (and cross-core launch skew) during phase 1 instead of stalling
        # the first real ReduceScatter by ~100us
        warm_in = dram.tile([1, 64], F16, name="warm_in")
        warm_out = dram.tile([4, 64], F16, name="warm_out")
        consts = top.enter_context(tc.tile_pool(name="consts", bufs=1))
        cos_sb = consts.tile([128, T], F16)
        sin_sb = consts.tile([128, T], F16)
        perm_sb = consts.tile([128, 128], F16)
        trim_sb = consts.tile([TK, 4 * TK], F16)
        bq_sb = consts.tile([128, NPAIRS], F32)
        bk_sb = consts.tile([128, NPAIRS], F32)
        bo_sb = consts.tile([128, KT], F32)
        sel_sb = consts.tile([NH, NPAIRS, 128], F16)
        bqr = bq.ap().rearrange("(p c) one -> c (p one)", c=128)
        bkr = bk.ap().rearrange("(p c) one -> c (p one)", c=128)
        bor = bo.ap().rearrange("(p c) one -> c (p one)", c=128)

        wo_pool = top.enter_context(tc.tile_pool(name="wo", bufs=1))
        wo_sb = wo_pool.tile([128, 2, C], F16)

        rqk_pool = top.enter_context(tc.tile_pool(name="rqk", bufs=1))
        # RQ/RK per pair: [128, T]; rows = (u1 h0, u2 h0, u1 h1, u2 h1) x 32
        RQ = [rqk_pool.tile([128, T], F16, name=f"RQ{p}") for p in range(NPAIRS)]
        RK = [rqk_pool.tile([128, T], F16, name=f"RK{p}") for p in range(NPAIRS)]
        v_pool = top.enter_context(tc.tile_pool(name="vsb", bufs=1))
        V_sb = v_pool.tile([128, NKT, HPG, HD + 1], F16)

        # ---------------- Phase 1: QKV projection + RoPE ----------------
        with ExitStack() as ph1:
            xt_pool = ph1.enter_context(tc.tile_pool(name="xt", bufs=1))
            xT_sb = xt_pool.tile([128, NQ, KT, TQ], F16)
            wqk_pool = ph1.enter_context(tc.tile_pool(name="wqk", bufs=1))
            wq_sb = wqk_pool.tile([128, KT, DGRP], F16)
            wk_sb = wqk_pool.tile([128, KT, DGRP], F16)
            wv_sb = wqk_pool.tile([128, KT, DGRP], F16)

            # DMA issue order = completion order: feed the PE's phase-1
            # consumption sequence. First q-chunk + wq arrive interleaved
            # per k-tile so the first accumulation chain starts early.
            warm_sb = consts.tile([1, 64], F16, name="warm_sb")
            nc.gpsimd.memset(warm_sb, 0.0)
            # dummy Ln+Exp first: bias the act-table chooser toward the
            # natural_log_exp table so per-chunk Ln/Exp pairs don't reload
            nc.scalar.activation(out=warm_sb, in_=warm_sb, func=AF.Ln)
            nc.scalar.activation(out=warm_sb, in_=warm_sb, func=AF.Exp)
            nc.gpsimd.dma_start(out=warm_in, in_=warm_sb)
            nc.gpsimd.collective_compute(
                "AllGather",
                ALU.bypass,
                ins=[warm_in.opt()],
                outs=[warm_out.opt()],
                replica_groups=[[0, 1, 2, 3], [4, 5, 6, 7]],
            )
            # inputs split across two queues: weights on gpsimd, x on sync,
            # interleaved per k-tile so the first accumulation starts early
            xrv = xr.ap().rearrange("p (n k t) -> p n k t", n=NQ, k=KT)
            wqv = wqr.ap().rearrange("p (k d) -> p k d", k=KT)
            for k in range(KT):
                nc.gpsimd.dma_start(out=wq_sb[:, k], in_=wqv[:, k])
                nc.sync.dma_start(out=xT_sb[:, 0, k], in_=xrv[:, 0, k])
            nc.gpsimd.dma_start(out=perm_sb, in_=perm.ap())
            nc.gpsimd.dma_start(out=bq_sb, in_=bqr)
            nc.gpsimd.dma_start(out=bk_sb, in_=bkr)
            nc.sync.dma_start(out=xT_sb[:, 1], in_=xrv[:, 1])
            nc.gpsimd.dma_start(out=wk_sb, in_=wkr.ap())
            nc.sync.dma_start(out=xT_sb[:, 2], in_=xrv[:, 2])
            nc.sync.dma_start(out=xT_sb[:, 3], in_=xrv[:, 3])
            nc.gpsimd.dma_start(out=cos_sb, in_=cos4.ap())
            nc.gpsimd.dma_start(out=sin_sb, in_=sin4.ap())
            nc.gpsimd.dma_start(out=wv_sb, in_=wvr.ap())
            nc.gpsimd.dma_start(out=trim_sb, in_=trim.ap())
            nc.gpsimd.dma_start(out=sel_sb, in_=sel.ap())
            nc.gpsimd.dma_start(out=bo_sb, in_=bor)
            nc.gpsimd.dma_start(out=wo_sb, in_=wor.ap())

            ps1 = ph1.enter_context(tc.tile_pool(name="ps1", bufs=2, space="PSUM"))
            tmp_pool = ph1.enter_context(tc.tile_pool(name="tmp", bufs=4))

            def rope_tail(Rc, n):
                # swap 32-row halves via PE perm matmul; sign baked in sin_sb
                sw_ps = ps1.tile([128, TQ], F32, name="sw_ps")
                nc.tensor.matmul(
                    out=sw_ps, lhsT=perm_sb, rhs=Rc, start=True, stop=True
                )
                tmpS = tmp_pool.tile([128, TQ], F16, name="tmpS")
                tmpC = tmp_pool.tile([128, TQ], F16, name="tmpC")
                nc.vector.tensor_mul(tmpS, sw_ps, sin_sb[:, n * TQ : (n + 1) * TQ])
                nc.vector.tensor_mul(tmpC, Rc, cos_sb[:, n * TQ : (n + 1) * TQ])
                nc.vector.tensor_add(Rc, tmpC, tmpS)

            pend = None
            for p in range(NPAIRS):
                for w_sb, b_sb, R in (
                    (wq_sb, bq_sb, RQ[p]),
                    (wk_sb, bk_sb, RK[p]),
                ):
                    for n in range(NQ):
                        u_ps = ps1.tile([128, TQ], F32, name="u_ps")
                        for k in range(KT):
                            nc.tensor.matmul(
                                out=u_ps,
                                lhsT=w_sb[:, k, p * 128 : (p + 1) * 128],
                                rhs=xT_sb[:, n, k, :],
                                start=(k == 0),
                                stop=(k == KT - 1),
                            )
                        Rc = R[:, n * TQ : (n + 1) * TQ]
                        # evict with bias -> R buffer (pre-rotation values)
                        nc.scalar.activation(
                            out=Rc,
                            in_=u_ps,
                            func=AF.Identity,
                            bias=b_sb[:, p : p + 1],
                        )
                        if pend is not None:
                            rope_tail(*pend)
                        pend = (Rc, n)
            rope_tail(*pend)

            # V tiles [t,d] with ones column per head
            nc.vector.memset(V_sb, 1.0)
            for tt in range(NKT):
                v_ps = ps1.tile([128, DGRP], F32, name="v_ps")
                for k in range(KT):
                    nc.tensor.matmul(
                        out=v_ps,
                        lhsT=xT_sb[:, tt // 4, k, (tt % 4) * TK : (tt % 4 + 1) * TK],
                        rhs=wv_sb[:, k, :],
                        start=(k == 0),
                        stop=(k == KT - 1),
                    )
                nc.vector.tensor_copy(
                    out=V_sb[:, tt, :, 0:HD],
                    in_=v_ps.rearrange("p (h d) -> p h d", h=HPG),
                )

        # ---------- Phase 2+3: attention, partial proj + ReduceScatter ----------
        with ExitStack() as ph2:
            sd_pool = ph2.enter_context(tc.tile_pool(name="sduo", bufs=4, space="PSUM"))
            av_pool = ph2.enter_context(tc.tile_pool(name="av", bufs=1, space="PSUM"))
            # bc + proj share one 2-deep ring (same tag): double-buffering
            # lets proj matmul mb+2 overlap the eviction of mb
            epi_ps = ph2.enter_context(tc.tile_pool(name="epi", bufs=2, space="PSUM"))
            pt_pool = ph2.enter_context(tc.tile_pool(name="ptile", bufs=4))
            o_pool = ph2.enter_context(tc.tile_pool(name="osb", bufs=2))
            yt_pool = ph2.enter_context(tc.tile_pool(name="yt", bufs=2))
            yp_pool = ph2.enter_context(tc.tile_pool(name="yp", bufs=3))
            dq_pool = ph2.enter_context(tc.tile_pool(name="dq", bufs=2))
            rb_pool = ph2.enter_context(tc.tile_pool(name="rb", bufs=2))
            av = [av_pool.tile([128, TQ], F32, name=f"av{hh}") for hh in range(2)]

            def issue_av(p, qi, ptile, ti, hh):
                last_ti = 4 * qi + 3
                # column-restrict masked diagonal blocks, except the
                # group-closing matmul (stop flag must cover the tile)
                off = TK * (ti - 4 * qi) if (4 * qi < ti < last_ti) else 0
                nc.tensor.matmul(
                    out=av[hh][0:65, off:TQ],
                    lhsT=V_sb[:, ti, 2 * p + hh, :],
                    rhs=ptile[:, (ti % 2) * TQ + off : (ti % 2 + 1) * TQ],
                    start=(ti == 0),
                    stop=(ti == last_ti),
                )

            def emit_epilogue(eqi, o_cur, rbuf):
                # PE part of chunk eqi's epilogue: recip broadcast + partial
                # projection; then evictions, DMA to y_part, ReduceScatter.
                yt2 = yt_pool.tile([128, NPAIRS, TQ], F16, name="yt2")
                for p in range(NPAIRS):
                    bc_ps = epi_ps.tile([128, TQ], F32, name="bc_ps", tag="epi")
                    nc.tensor.matmul(
                        out=bc_ps,
                        lhsT=sel_sb[:, p, :],
                        rhs=rbuf,
                        start=True,
                        stop=True,
                    )
                    nc.vector.tensor_mul(yt2[:, p, :], o_cur[:, p, :], bc_ps)
                y_part = y_part_q[eqi]
                for mb in range(KT):
                    o_ps = epi_ps.tile([128, TQ], F32, name="o_ps", tag="epi")
                    for p in range(NPAIRS):
                        nc.tensor.matmul(
                            out=o_ps,
                            lhsT=wo_sb[:, p, mb * 128 : (mb + 1) * 128],
                            rhs=yt2[:, p, :],
                            start=(p == 0),
                            stop=(p == NPAIRS - 1),
                        )
                    yp = yp_pool.tile([128, TQ], F16, name="yp")
                    nc.vector.tensor_scalar_add(yp, o_ps, bo_sb[:, mb : mb + 1])
                    nc.sync.dma_start(out=y_part[mb], in_=yp)
                nc.gpsimd.collective_compute(
                    "ReduceScatter",
                    ALU.add,
                    ins=[y_part.opt()],
                    outs=[y_rs_q[eqi].opt()],
                    replica_groups=[[0, 1, 2, 3], [4, 5, 6, 7]],
                )

            pending = None
            for qi in range(NQ):
                o_cur = o_pool.tile([128, NPAIRS, TQ], F32, name="o_cur")
                dq = dq_pool.tile([1, NH, TQ], F16, name="dq")
                for p in range(NPAIRS):
                    avq = []
                    units = 0
                    for g2 in range(2 * qi + 2):
                        for hh in range(2):
                            diag = g2 >= 2 * qi
                            ptile = pt_pool.tile([128, 2 * TQ], F16, name="ptile")
                            for ji in range(2):
                                ti = 2 * g2 + ji
                                off = TK * (ti - 4 * qi) if diag and ti > 4 * qi else 0
                                col = ji * TQ
                                s_u = sd_pool.tile([128, TQ], F32, name="s_u")
                                nc.tensor.matmul(
                                    out=s_u[:, off:TQ],
                                    lhsT=RK[p][
                                        64 * hh : 64 * hh + 64,
                                        ti * TK : (ti + 1) * TK,
                                    ],
                                    rhs=RQ[p][
                                        64 * hh : 64 * hh + 64,
                                        qi * TQ + off : (qi + 1) * TQ,
                                    ],
                                    start=True,
                                    stop=True,
                                )
                                nc.scalar.activation(
                                    out=ptile[:, col + off : col + TQ],
                                    in_=s_u[:, off:TQ],
                                    func=AF.Exp,
                                    scale=SCALE,
                                )
                                if diag:
                                    # zero masked cols + upper-tri the
                                    # diagonal 128-block in one mask mul
                                    # (stale cols [col, col+off) x 0 = 0)
                                    nc.vector.tensor_mul(
                                        ptile[:, col : col + off + TK],
                                        ptile[:, col : col + off + TK],
                                        trim_sb[:, 3 * TK - off : 4 * TK],
                                    )
                                avq.append((ptile, ti, hh))
                                # AV lags S by 3 units so the PE never
                                # waits on the exp of its own unit
                                if len(avq) > 3:
                                    issue_av(p, qi, *avq.pop(0))
                                units += 1
                                if pending is not None and p == 0 and units == 8:
                                    emit_epilogue(*pending)
                                    pending = None
                    for u in avq:
                        issue_av(p, qi, *u)

                    # evict O_aug + denominators for this (p, qi)
                    nc.vector.tensor_copy(out=o_cur[0:64, p, :], in_=av[0][0:64, :])
                    nc.vector.tensor_copy(
                        out=o_cur[64:128, p, :], in_=av[1][0:64, :]
                    )
                    # engine partition bases must be 32-aligned, so stage
                    # denom rows in free dim of one partition, scatter via DMA
                    nc.vector.tensor_copy(out=dq[0:1, 2 * p, :], in_=av[0][64:65, :])
                    nc.vector.tensor_copy(
                        out=dq[0:1, 2 * p + 1, :], in_=av[1][64:65, :]
                    )

                # normalize prep: scatter denoms to NH partitions, 1/d =
                # exp(-ln d) on ACT. (ACT Reciprocal is blocked by bass for
                # accuracy; DVE InstReciprocal blocks the Vector queue
                # ~12us; DVE divide fails ISA codegen.)
                rbuf = rb_pool.tile([NH, TQ], F16, name="rbuf")
                nc.gpsimd.dma_start(out=rbuf, in_=dq[0:1, :, :])
                nc.scalar.activation(out=rbuf, in_=rbuf, func=AF.Ln)
                nc.scalar.activation(out=rbuf, in_=rbuf, func=AF.Exp, scale=-1.0)
                pending = (qi, o_cur, rbuf)
            emit_epilogue(*pending)
            # out-copies last: a copy waiting on its RS must never block
            # later eviction DMAs behind it in the queue
            for qi in range(NQ):
                nc.sync.dma_start(out=outs_q[qi].ap(), in_=y_rs_q[qi].opt())
    nc.finalize()
    return nc


_NC = None


def _get_nc():
    global _NC
    if _NC is None:
        _NC = build_nc()
    return _NC


def _relay(w):
    # [KT*128, M] -> [128, KT*M] so each partition's DMA line is contiguous
    kt, m = w.shape[0] // 128, w.shape[1]
    return np.ascontiguousarray(
        w.reshape(kt, 128, m).transpose(1, 0, 2).reshape(128, kt * m)
    )


def _relay_x(xb):
    # x^T [C, T] -> [128, NQ*KT*TQ] n-major so each 512-col block is one
    # contiguous-per-partition DMA
    xt = xb.T.reshape(KT, 128, NQ, TQ)
    return np.ascontiguousarray(xt.transpose(1, 2, 0, 3).reshape(128, NQ * KT * TQ))


def _in_maps(x, freqs_cos, freqs_sin, Wqkv, bqkv, Wproj, bproj):
    f16 = np.float16
    x = np.asarray(x, f16)
    Wqkv = np.asarray(Wqkv, f16)
    bqkv = np.asarray(bqkv, np.float32)
    Wproj32 = np.asarray(Wproj, np.float32)
    Wproj = Wproj32.astype(f16)
    bproj = np.asarray(bproj, np.float32)
    cos4 = np.ascontiguousarray(np.tile(np.asarray(freqs_cos, f16).T, (4, 1)))
    sinT = np.asarray(freqs_sin, f16).T  # [32, T]
    sin4 = np.ascontiguousarray(np.tile(np.concatenate([-sinT, sinT], axis=0), (2, 1)))
    perm = np.zeros((128, 128), f16)
    for j in range(128):
        i = j + 32 if (j % 64) < 32 else j - 32
        perm[i, j] = 1.0
    trim = np.zeros((TK, 4 * TK), f16)
    trim[:, 3 * TK :] = np.triu(np.ones((TK, TK), f16))
    sel = np.zeros((NH, NPAIRS, 128), f16)
    for p in range(NPAIRS):
        sel[2 * p, p, 0:64] = 1.0
        sel[2 * p + 1, p, 64:128] = 1.0
    bproj_eff = (bproj + bqkv[2 * C : 3 * C] @ Wproj32) / GROUPS
    maps = []
    for r in range(NCORES):
        b, g = r // GROUPS, r % GROUPS
        sl = slice(DGRP * g, DGRP * (g + 1))
        maps.append(
            {
                "xr": _relay_x(x[b]),
                "wqr": _relay(Wqkv[:, 0 * C :][:, sl]),
                "wkr": _relay(Wqkv[:, 1 * C :][:, sl]),
                "wvr": _relay(Wqkv[:, 2 * C :][:, sl]),
                "wor": _relay(Wproj[sl, :]),
                "cos4": cos4,
                "sin4": sin4,
                "perm": perm,
                "trim": trim,
                "sel": sel,
                "bq": np.ascontiguousarray(
                    bqkv[0 * C : 1 * C][sl], np.float32
                ).reshape(DGRP, 1),
                "bk": np.ascontiguousarray(
                    bqkv[1 * C : 2 * C][sl], np.float32
                ).reshape(DGRP, 1),
                "bo": np.ascontiguousarray(bproj_eff, np.float32).reshape(C, 1),
            }
        )
    return maps


def _assemble(results):
    y = np.empty((B, T, C), np.float32)
    for b in range(B):
        cat = np.concatenate(
            [
                np.concatenate(
                    [
                        np.asarray(results[GROUPS * b + g][f"out{q}"])
                        for q in range(NQ)
                    ],
                    axis=1,
                )
                for g in range(GROUPS)
            ],
            axis=0,
        )
        y[b] = cat.T.astype(np.float32)
    return y


def kernel(**inputs):
    nc = _get_nc()
    res = run_bass_kernel_spmd(nc, _in_maps(**inputs), core_ids=list(range(NCORES)))
    return _assemble(res.results)


def kernel_traced(**inputs):
    import tempfile

    nc = _get_nc()
    tmpdir = tempfile.mkdtemp(prefix="mha_trace_")
    res = run_bass_kernel_spmd(
        nc,
        _in_maps(**inputs),
        core_ids=list(range(NCORES)),
        trace=True,
        trace_cores=list(range(NCORES)),
        tmpdir=tmpdir,
    )
    return _assemble(res.results), res.exec_time_ns, tmpdir
